# revision 13
# baseline (speedup 1.0000x reference)
"""Causal attention (B=8, S=2048, D=128, f32) on 8 TRN2 NeuronCores.

Strategy: batch-parallel SPMD — each core computes full causal attention for
one batch element.

Per-core algorithm (layouts chosen so softmax/PV need no on-chip transposes):
  - Host passes Q^T, K^T as [D=128, S=2048] f32 (D on partitions) and V
    pre-arranged as VS [128, S] bf16 where column block j holds V rows
    [128j, 128j+128).
  - Scores are computed transposed, per key block j:
        S^T_j[k, q] = (K^T_j)-stationary.T @ Q^T-moving   (PSUM, f32)
    float32r matmuls (1 cycle/row for moving width >= 256) keep the scores
    at full precision (exp amplifies score error).
  - Causal mask applied additively on the PSUM scores (diagonal block only).
  - exp with the 1/sqrt(D) scale folded into ScalarE's activation affine,
    PSUM -> SBUF, output in bf16 (P^T tiles).
  - out^T[d, q] += V_j-stationary @ P^T_j-moving (bf16 in, f32 accumulate).
  - rowsum[q]  += ones-stationary @ P^T_j-moving (M=1 matmul, bf16).
  - Normalize per 512-wide q-chunk as soon as its accumulation finishes:
    evacuate out^T chunk, broadcast rowsum across partitions with a K=1
    matmul (reusing the freed out^T PSUM bank in place),
    reciprocal_approx_fast, multiply, DMA out.
  - Host transposes out^T back to [S, D].

TensorE work is software-pipelined: scores for key block j+1 are emitted
before PV/rowsum of block j so the PE never head-of-line blocks on ScalarE's
exp. The q axis is processed in two passes of 1024 so PSUM fits:
  staging S^T [128,1024] x2 bufs (4 banks) + out^T [128,1024] (2 banks)
  + 2x rowsum [1,512] (2 banks) = 8 banks.
"""

import math
import sys

import numpy as np
import ml_dtypes

sys.path.insert(0, "/opt/trn_rl_repo")

from concourse import bacc, mybir
from concourse.bass_utils import run_bass_kernel_spmd
from concourse.tile import TileContext

F32 = mybir.dt.float32
F32R = mybir.dt.float32r
BF16 = mybir.dt.bfloat16
BF16_NP = np.dtype(ml_dtypes.bfloat16)

B, S, D = 8, 2048, 128
NBLK = S // 128  # 16 key blocks
HALF = 1024  # q-pass width
SCALE = 1.0 / math.sqrt(D)
MASKNEG = -1e30

_NC_CACHE = None
_ONES = np.ones((128, 128), dtype=np.float32)


def _chunks_for_block(j, q0):
    """Matmul chunks for key block j in pass [q0, q0+HALF): list of
    (a, b, h) global q ranges clipped to psum bank h (bf16: no min width)."""
    k0 = 128 * j
    q_lo = max(q0, k0)
    out = []
    for h in range(2):
        a = max(q_lo, q0 + 512 * h)
        b = q0 + 512 * (h + 1)
        if a < b:
            out.append((a, b, h))
    return out


def _build_nc():
    nc = bacc.Bacc("TRN2", target_bir_lowering=False, debug=False, num_devices=8)

    qt_d = nc.dram_tensor("QT", [D, S], BF16, kind="ExternalInput")
    kt_d = nc.dram_tensor("KT", [D, S], BF16, kind="ExternalInput")
    vs_d = nc.dram_tensor("VS", [128, S], BF16, kind="ExternalInput")
    ones_d = nc.dram_tensor("ONES", [128, 128], F32R, kind="ExternalInput")
    out_d = nc.dram_tensor("out", [D, S], F32, kind="ExternalOutput")

    with TileContext(nc) as tc:
        with (
            tc.tile_pool(name="persist", bufs=1) as persist,
            tc.tile_pool(name="ptp", bufs=4) as ptp,
            tc.tile_pool(name="epi", bufs=2) as epi,
            tc.tile_pool(name="spool", bufs=2, space="PSUM") as spool,
            tc.tile_pool(name="opool", bufs=1, space="PSUM") as opool,
            tc.tile_pool(name="rpool", bufs=2, space="PSUM") as rpool,
        ):
            qt = persist.tile([D, S], BF16, tag="qt")
            kt = persist.tile([D, S], BF16, tag="kt")
            vs = persist.tile([128, S], BF16, tag="vs")  # col block j = V rows
            ones = persist.tile([128, 128], F32R, tag="ones")
            ones_r = ones[0:1, :]
            ones_b = persist.tile([128, 1], BF16, tag="ones_b")

            # additive causal mask, f32: strict-lower-triangle = MASKNEG
            mask = persist.tile([128, 128], F32, tag="mask")
            nc.gpsimd.memset(mask[:, :], 0.0)
            nc.gpsimd.affine_select(
                out=mask[:, :],
                in_=mask[:, :],
                compare_op=mybir.AluOpType.is_ge,
                fill=MASKNEG,
                base=0,
                pattern=[[1, 128]],
                channel_multiplier=-1,
            )

            # warm the ScalarE exp table while input DMAs run
            warm = epi.tile([1, 16], F32, tag="warm")
            nc.scalar.activation(
                warm[:, :],
                mask[0:1, 0:16],
                mybir.ActivationFunctionType.Exp,
                scale=SCALE,
            )

            # ---- input DMAs: critical prefix split across the SP hw queue
            # and gpsimd's software DGE; bulk on gpsimd in first-need order --
            nc.sync.dma_start(kt[:, 0:128], kt_d[:, 0:128])
            nc.sync.dma_start(qt[:, 0:256], qt_d[:, 0:256])
            nc.sync.dma_start(qt[:, 512:768], qt_d[:, 512:768])
            nc.sync.dma_start(vs[:, 0:512], vs_d[:, 0:512])
            nc.sync.dma_start(ones[:, :], ones_d[:, :])
            nc.gpsimd.dma_start(qt[:, 256:512], qt_d[:, 256:512])
            nc.gpsimd.dma_start(qt[:, 768:1024], qt_d[:, 768:1024])
            with nc.allow_low_precision(reason="exact ones to bf16"):
                nc.vector.tensor_copy(ones_b[:, :], ones[:, 0:1])
            for c in range(1, 8):  # kt blocks 1..7
                nc.gpsimd.dma_start(kt[:, c * 128 : (c + 1) * 128],
                                    kt_d[:, c * 128 : (c + 1) * 128])
            nc.gpsimd.dma_start(vs[:, 512:1024], vs_d[:, 512:1024])
            for c in range(4, 6):  # qt q in [1024, 1536)
                nc.gpsimd.dma_start(qt[:, c * 256 : (c + 1) * 256],
                                    qt_d[:, c * 256 : (c + 1) * 256])
            for c in range(8, 12):  # kt blocks 8..11
                nc.gpsimd.dma_start(kt[:, c * 128 : (c + 1) * 128],
                                    kt_d[:, c * 128 : (c + 1) * 128])
            nc.gpsimd.dma_start(vs[:, 1024:1536], vs_d[:, 1024:1536])
            for c in range(6, 8):  # qt q in [1536, 2048)
                nc.gpsimd.dma_start(qt[:, c * 256 : (c + 1) * 256],
                                    qt_d[:, c * 256 : (c + 1) * 256])
            for c in range(12, 16):  # kt blocks 12..15
                nc.gpsimd.dma_start(kt[:, c * 128 : (c + 1) * 128],
                                    kt_d[:, c * 128 : (c + 1) * 128])
            nc.gpsimd.dma_start(vs[:, 1536:2048], vs_d[:, 1536:2048])

            for qh in range(2):
                q0 = qh * HALF  # global q offset of this pass
                njb = (q0 + HALF) // 128  # key blocks this pass

                out_ps = opool.tile([D, HALF], F32, tag="outps",
                                    name=f"outps_{qh}")
                rs = [
                    rpool.tile([1, 512], F32, tag="rs", name=f"rs_{qh}_{h}")
                    for h in range(2)
                ]
                # last key block that touches each 512-half of out_ps
                j_last = [(q0 + 512 * (h + 1)) // 128 - 1 for h in range(2)]

                def emit_scores(j, qh=qh, q0=q0):
                    """QK^T chunks + mask + per-chunk exp for key block j.
                    exp is emitted per 512-chunk so PV of one chunk never
                    waits the other chunk's scores or the diagonal mask."""
                    k0 = 128 * j
                    chunks = _chunks_for_block(j, q0)

                    sps = spool.tile([128, HALF], F32, tag="sps",
                                     name=f"sps_{qh}_{j}")
                    pt = ptp.tile([128, HALF], BF16, tag="pt",
                                  name=f"pt_{qh}_{j}")
                    for (a, b, _h) in chunks:
                        if j == 0 and qh == 0 and a == 0:
                            # split the very first matmul so PE starts as
                            # soon as the first 256 qt columns arrive
                            nc.tensor.matmul(sps[:, 0:256], kt[:, 0:128],
                                             qt[:, 0:256], start=True,
                                             stop=True)
                            nc.tensor.matmul(sps[:, 256:512], kt[:, 0:128],
                                             qt[:, 256:512], start=True,
                                             stop=True)
                        else:
                            nc.tensor.matmul(
                                sps[:, a - q0 : b - q0],
                                kt[:, k0 : k0 + 128],
                                qt[:, a:b],
                                start=True,
                                stop=True,
                            )
                    for (a, b, _h) in chunks:
                        if k0 >= a and k0 < b:
                            dl = k0 - q0
                            nc.vector.tensor_add(
                                sps[:, dl : dl + 128],
                                sps[:, dl : dl + 128],
                                mask[:, :],
                            )
                        nc.scalar.activation(
                            pt[:, a - q0 : b - q0],
                            sps[:, a - q0 : b - q0],
                            mybir.ActivationFunctionType.Exp,
                            scale=SCALE,
                        )
                    return pt

                def emit_pv(j, pt, q0=q0):
                    """PV + rowsum accumulation for key block j. Chunks are
                    grouped per stationary operand: consecutive matmuls with
                    the same weights skip the ~107ns weight-reload stall."""
                    k0 = 128 * j
                    chunks = _chunks_for_block(j, q0)
                    for (a, b, h) in chunks:
                        nc.tensor.matmul(
                            out_ps[:, a - q0 : b - q0],
                            vs[:, k0 : k0 + 128],
                            pt[:, a - q0 : b - q0],
                            start=(j == 0),
                            stop=(j == j_last[h]),
                        )
                    for (a, b, h) in chunks:
                        nc.tensor.matmul(
                            rs[h][:, a - (q0 + 512 * h) : b - (q0 + 512 * h)],
                            ones_b[:, :],
                            pt[:, a - q0 : b - q0],
                            start=(j == 0),
                            stop=(j == j_last[h]),
                        )

                def emit_epi_half(h, qh=qh, q0=q0):
                    """Normalize + store q-chunk [q0+512h, q0+512h+512)."""
                    sl = slice(512 * h, 512 * (h + 1))
                    rs_row = epi.tile([1, 512], F32R, tag="rs_row",
                                      name=f"rsr_{qh}_{h}")
                    with nc.allow_low_precision(reason="f32r softmax denom"):
                        nc.vector.tensor_copy(rs_row[:, :], rs[h][:, :])
                    o_evac = epi.tile([D, 512], F32, tag="o_evac",
                                      name=f"oev_{qh}_{h}")
                    nc.vector.tensor_copy(o_evac[:, :], out_ps[:, sl])
                    # broadcast rowsum into the just-freed out_ps bank
                    nc.tensor.matmul(
                        out_ps[:, sl],
                        ones_r[:, :],
                        rs_row[:, :],
                        start=True,
                        stop=True,
                    )
                    rb = epi.tile([128, 512], F32, tag="rb",
                                  name=f"rb_{qh}_{h}")
                    nc.vector.reciprocal_approx_fast(
                        out=rb[:, :], in_=out_ps[:, sl]
                    )
                    o_fin = epi.tile([D, 512], F32, tag="o_fin",
                                     name=f"ofin_{qh}_{h}")
                    nc.vector.tensor_mul(o_fin[:, :], o_evac[:, :], rb[:, :])
                    for c in range(4):
                        nc.sync.dma_start(
                            out_d[:, q0 + 512 * h + 128 * c :
                                  q0 + 512 * h + 128 * (c + 1)],
                            o_fin[:, 128 * c : 128 * (c + 1)],
                        )

                # software pipeline: scores for j+1 before PV of j;
                # per-half epilogue as soon as its accumulation completes
                pts = {0: emit_scores(0)}
                for j in range(njb):
                    if j + 1 < njb:
                        pts[j + 1] = emit_scores(j + 1)
                    emit_pv(j, pts.pop(j))
                    for h in range(2):
                        if j == j_last[h]:
                            emit_epi_half(h)

    nc.compile()
    return nc


def _get_nc():
    global _NC_CACHE
    if _NC_CACHE is None:
        _NC_CACHE = _build_nc()
    return _NC_CACHE


def _in_maps(Q, K, V):
    maps = []
    for b in range(B):
        vsb = np.ascontiguousarray(
            V[b].reshape(NBLK, 128, D).transpose(1, 0, 2).reshape(128, S)
        ).astype(BF16_NP)
        maps.append(
            {
                "QT": np.ascontiguousarray(Q[b].T).astype(BF16_NP),
                "KT": np.ascontiguousarray(K[b].T).astype(BF16_NP),
                "VS": vsb,
                "ONES": _ONES,
            }
        )
    return maps


def kernel(Q, K, V):
    Q = np.asarray(Q, dtype=np.float32)
    K = np.asarray(K, dtype=np.float32)
    V = np.asarray(V, dtype=np.float32)
    assert Q.shape == (B, S, D), Q.shape

    nc = _get_nc()
    res = run_bass_kernel_spmd(nc, _in_maps(Q, K, V), core_ids=list(range(B)))
    return np.stack(
        [np.ascontiguousarray(res.results[b]["out"].T) for b in range(B)], axis=0
    )


# revision 14
# speedup vs baseline: 1.0203x; 1.0203x over previous
"""Causal attention (B=8, S=2048, D=128, f32) on 8 TRN2 NeuronCores.

Strategy: batch-parallel SPMD — each core computes full causal attention for
one batch element.

Per-core algorithm (layouts chosen so softmax/PV need no on-chip transposes):
  - Host passes Q^T, K^T as [D=128, S=2048] f32 (D on partitions) and V
    pre-arranged as VS [128, S] bf16 where column block j holds V rows
    [128j, 128j+128).
  - Scores are computed transposed, per key block j:
        S^T_j[k, q] = (K^T_j)-stationary.T @ Q^T-moving   (PSUM, f32)
    float32r matmuls (1 cycle/row for moving width >= 256) keep the scores
    at full precision (exp amplifies score error).
  - Causal mask applied additively on the PSUM scores (diagonal block only).
  - exp with the 1/sqrt(D) scale folded into ScalarE's activation affine,
    PSUM -> SBUF, output in bf16 (P^T tiles).
  - out^T[d, q] += V_j-stationary @ P^T_j-moving (bf16 in, f32 accumulate).
  - rowsum[q]  += ones-stationary @ P^T_j-moving (M=1 matmul, bf16).
  - Normalize per 512-wide q-chunk as soon as its accumulation finishes:
    evacuate out^T chunk, broadcast rowsum across partitions with a K=1
    matmul (reusing the freed out^T PSUM bank in place),
    reciprocal_approx_fast, multiply, DMA out.
  - Host transposes out^T back to [S, D].

TensorE work is software-pipelined: scores for key block j+1 are emitted
before PV/rowsum of block j so the PE never head-of-line blocks on ScalarE's
exp. The q axis is processed in two passes of 1024 so PSUM fits:
  staging S^T [128,1024] x2 bufs (4 banks) + out^T [128,1024] (2 banks)
  + 2x rowsum [1,512] (2 banks) = 8 banks.
"""

import math
import sys

import numpy as np
import ml_dtypes

sys.path.insert(0, "/opt/trn_rl_repo")

from concourse import bacc, mybir
from concourse.bass_utils import run_bass_kernel_spmd
from concourse.tile import TileContext

F32 = mybir.dt.float32
F32R = mybir.dt.float32r
BF16 = mybir.dt.bfloat16
BF16_NP = np.dtype(ml_dtypes.bfloat16)

B, S, D = 8, 2048, 128
NBLK = S // 128  # 16 key blocks
HALF = 1024  # q-pass width
SCALE = 1.0 / math.sqrt(D)
MASKNEG = -1e30

_NC_CACHE = None
_ONES = np.ones((128, 128), dtype=np.float32)


def _chunks_for_block(j, q0):
    """Matmul chunks for key block j in pass [q0, q0+HALF): list of
    (a, b, h) global q ranges clipped to psum bank h (bf16: no min width)."""
    k0 = 128 * j
    q_lo = max(q0, k0)
    out = []
    for h in range(2):
        a = max(q_lo, q0 + 512 * h)
        b = q0 + 512 * (h + 1)
        if a < b:
            out.append((a, b, h))
    return out


def _build_nc():
    nc = bacc.Bacc("TRN2", target_bir_lowering=False, debug=False, num_devices=8)

    qt_d = nc.dram_tensor("QT", [D, S], BF16, kind="ExternalInput")
    kt_d = nc.dram_tensor("KT", [D, S], BF16, kind="ExternalInput")
    vs_d = nc.dram_tensor("VS", [128, S], BF16, kind="ExternalInput")
    ones_d = nc.dram_tensor("ONES", [128, 128], F32R, kind="ExternalInput")
    out_d = nc.dram_tensor("out", [D, S], F32, kind="ExternalOutput")

    with TileContext(nc) as tc:
        with (
            tc.tile_pool(name="persist", bufs=1) as persist,
            tc.tile_pool(name="ptp", bufs=4) as ptp,
            tc.tile_pool(name="epi", bufs=2) as epi,
            tc.tile_pool(name="spool", bufs=2, space="PSUM") as spool,
            tc.tile_pool(name="opool", bufs=1, space="PSUM") as opool,
            tc.tile_pool(name="rpool", bufs=2, space="PSUM") as rpool,
        ):
            qt = persist.tile([D, S], BF16, tag="qt")
            kt = persist.tile([D, S], BF16, tag="kt")
            vs = persist.tile([128, S], BF16, tag="vs")  # col block j = V rows
            ones = persist.tile([128, 128], F32R, tag="ones")
            ones_r = ones[0:1, :]
            ones_b = persist.tile([128, 1], BF16, tag="ones_b")

            # additive causal mask, f32: strict-lower-triangle = MASKNEG
            mask = persist.tile([128, 128], F32, tag="mask")
            nc.gpsimd.memset(mask[:, :], 0.0)
            nc.gpsimd.affine_select(
                out=mask[:, :],
                in_=mask[:, :],
                compare_op=mybir.AluOpType.is_ge,
                fill=MASKNEG,
                base=0,
                pattern=[[1, 128]],
                channel_multiplier=-1,
            )

            # warm the ScalarE exp table while input DMAs run
            warm = epi.tile([1, 16], F32, tag="warm")
            nc.scalar.activation(
                warm[:, :],
                mask[0:1, 0:16],
                mybir.ActivationFunctionType.Exp,
                scale=SCALE,
            )

            # ---- input DMAs: critical prefix split across the SP hw queue
            # and gpsimd's software DGE; bulk on gpsimd in first-need order --
            nc.sync.dma_start(kt[:, 0:128], kt_d[:, 0:128])
            nc.sync.dma_start(qt[:, 0:256], qt_d[:, 0:256])
            nc.sync.dma_start(qt[:, 512:768], qt_d[:, 512:768])
            nc.sync.dma_start(vs[:, 0:512], vs_d[:, 0:512])
            nc.sync.dma_start(ones[:, :], ones_d[:, :])
            nc.gpsimd.dma_start(qt[:, 256:512], qt_d[:, 256:512])
            nc.gpsimd.dma_start(qt[:, 768:1024], qt_d[:, 768:1024])
            with nc.allow_low_precision(reason="exact ones to bf16"):
                nc.vector.tensor_copy(ones_b[:, :], ones[:, 0:1])
            for c in range(1, 8):  # kt blocks 1..7
                nc.gpsimd.dma_start(kt[:, c * 128 : (c + 1) * 128],
                                    kt_d[:, c * 128 : (c + 1) * 128])
            nc.gpsimd.dma_start(vs[:, 512:1024], vs_d[:, 512:1024])
            for c in range(4, 6):  # qt q in [1024, 1536)
                nc.gpsimd.dma_start(qt[:, c * 256 : (c + 1) * 256],
                                    qt_d[:, c * 256 : (c + 1) * 256])
            for c in range(8, 12):  # kt blocks 8..11
                nc.gpsimd.dma_start(kt[:, c * 128 : (c + 1) * 128],
                                    kt_d[:, c * 128 : (c + 1) * 128])
            nc.gpsimd.dma_start(vs[:, 1024:1536], vs_d[:, 1024:1536])
            for c in range(6, 8):  # qt q in [1536, 2048)
                nc.gpsimd.dma_start(qt[:, c * 256 : (c + 1) * 256],
                                    qt_d[:, c * 256 : (c + 1) * 256])
            for c in range(12, 16):  # kt blocks 12..15
                nc.gpsimd.dma_start(kt[:, c * 128 : (c + 1) * 128],
                                    kt_d[:, c * 128 : (c + 1) * 128])
            nc.gpsimd.dma_start(vs[:, 1536:2048], vs_d[:, 1536:2048])

            for qh in range(2):
                q0 = qh * HALF  # global q offset of this pass
                njb = (q0 + HALF) // 128  # key blocks this pass

                out_ps = opool.tile([D, HALF], F32, tag="outps",
                                    name=f"outps_{qh}")
                rs = [
                    rpool.tile([1, 512], F32, tag="rs", name=f"rs_{qh}_{h}")
                    for h in range(2)
                ]
                # last key block that touches each 512-half of out_ps
                j_last = [(q0 + 512 * (h + 1)) // 128 - 1 for h in range(2)]

                def emit_scores(j, qh=qh, q0=q0):
                    """QK^T chunks + mask + per-chunk exp for key block j.
                    exp is emitted per 512-chunk so PV of one chunk never
                    waits the other chunk's scores or the diagonal mask."""
                    k0 = 128 * j
                    chunks = _chunks_for_block(j, q0)

                    sps = spool.tile([128, HALF], F32, tag="sps",
                                     name=f"sps_{qh}_{j}")
                    pt = ptp.tile([128, HALF], BF16, tag="pt",
                                  name=f"pt_{qh}_{j}")
                    for (a, b, _h) in chunks:
                        if j == 0 and qh == 0 and a == 0:
                            # split the very first matmul so PE starts as
                            # soon as the first 256 qt columns arrive
                            nc.tensor.matmul(sps[:, 0:256], kt[:, 0:128],
                                             qt[:, 0:256], start=True,
                                             stop=True)
                            nc.tensor.matmul(sps[:, 256:512], kt[:, 0:128],
                                             qt[:, 256:512], start=True,
                                             stop=True)
                        else:
                            nc.tensor.matmul(
                                sps[:, a - q0 : b - q0],
                                kt[:, k0 : k0 + 128],
                                qt[:, a:b],
                                start=True,
                                stop=True,
                            )
                    if k0 >= q0:
                        dl = k0 - q0
                        nc.vector.tensor_add(
                            sps[:, dl : dl + 128],
                            sps[:, dl : dl + 128],
                            mask[:, :],
                        )
                    lo = chunks[0][0]
                    nc.scalar.activation(
                        pt[:, lo - q0 : HALF],
                        sps[:, lo - q0 : HALF],
                        mybir.ActivationFunctionType.Exp,
                        scale=SCALE,
                    )
                    return pt

                def emit_pv(j, pt, q0=q0):
                    """PV + rowsum accumulation for key block j. Chunks are
                    grouped per stationary operand: consecutive matmuls with
                    the same weights skip the ~107ns weight-reload stall."""
                    k0 = 128 * j
                    chunks = _chunks_for_block(j, q0)
                    for (a, b, h) in chunks:
                        nc.tensor.matmul(
                            out_ps[:, a - q0 : b - q0],
                            vs[:, k0 : k0 + 128],
                            pt[:, a - q0 : b - q0],
                            start=(j == 0),
                            stop=(j == j_last[h]),
                        )
                    for (a, b, h) in chunks:
                        nc.tensor.matmul(
                            rs[h][:, a - (q0 + 512 * h) : b - (q0 + 512 * h)],
                            ones_b[:, :],
                            pt[:, a - q0 : b - q0],
                            start=(j == 0),
                            stop=(j == j_last[h]),
                        )

                def emit_epi_half(h, qh=qh, q0=q0):
                    """Normalize + store q-chunk [q0+512h, q0+512h+512)."""
                    sl = slice(512 * h, 512 * (h + 1))
                    rs_row = epi.tile([1, 512], F32R, tag="rs_row",
                                      name=f"rsr_{qh}_{h}")
                    with nc.allow_low_precision(reason="f32r softmax denom"):
                        nc.vector.tensor_copy(rs_row[:, :], rs[h][:, :])
                    o_evac = epi.tile([D, 512], F32, tag="o_evac",
                                      name=f"oev_{qh}_{h}")
                    nc.vector.tensor_copy(o_evac[:, :], out_ps[:, sl])
                    # broadcast rowsum into the just-freed out_ps bank
                    nc.tensor.matmul(
                        out_ps[:, sl],
                        ones_r[:, :],
                        rs_row[:, :],
                        start=True,
                        stop=True,
                    )
                    rb = epi.tile([128, 512], F32, tag="rb",
                                  name=f"rb_{qh}_{h}")
                    nc.vector.reciprocal_approx_fast(
                        out=rb[:, :], in_=out_ps[:, sl]
                    )
                    o_fin = epi.tile([D, 512], F32, tag="o_fin",
                                     name=f"ofin_{qh}_{h}")
                    nc.vector.tensor_mul(o_fin[:, :], o_evac[:, :], rb[:, :])
                    for c in range(4):
                        nc.sync.dma_start(
                            out_d[:, q0 + 512 * h + 128 * c :
                                  q0 + 512 * h + 128 * (c + 1)],
                            o_fin[:, 128 * c : 128 * (c + 1)],
                        )

                # software pipeline: scores for j+1 before PV of j;
                # per-half epilogue as soon as its accumulation completes
                pts = {0: emit_scores(0)}
                for j in range(njb):
                    if j + 1 < njb:
                        pts[j + 1] = emit_scores(j + 1)
                    emit_pv(j, pts.pop(j))
                    for h in range(2):
                        if j == j_last[h]:
                            emit_epi_half(h)

    nc.compile()
    return nc


def _get_nc():
    global _NC_CACHE
    if _NC_CACHE is None:
        _NC_CACHE = _build_nc()
    return _NC_CACHE


def _in_maps(Q, K, V):
    maps = []
    for b in range(B):
        vsb = np.ascontiguousarray(
            V[b].reshape(NBLK, 128, D).transpose(1, 0, 2).reshape(128, S)
        ).astype(BF16_NP)
        maps.append(
            {
                "QT": np.ascontiguousarray(Q[b].T).astype(BF16_NP),
                "KT": np.ascontiguousarray(K[b].T).astype(BF16_NP),
                "VS": vsb,
                "ONES": _ONES,
            }
        )
    return maps


def kernel(Q, K, V):
    Q = np.asarray(Q, dtype=np.float32)
    K = np.asarray(K, dtype=np.float32)
    V = np.asarray(V, dtype=np.float32)
    assert Q.shape == (B, S, D), Q.shape

    nc = _get_nc()
    res = run_bass_kernel_spmd(nc, _in_maps(Q, K, V), core_ids=list(range(B)))
    return np.stack(
        [np.ascontiguousarray(res.results[b]["out"].T) for b in range(B)], axis=0
    )


# revision 15
# speedup vs baseline: 1.0523x; 1.0314x over previous
"""Causal attention (B=8, S=2048, D=128, f32) on 8 TRN2 NeuronCores.

Strategy: batch-parallel SPMD — each core computes full causal attention for
one batch element.

Per-core algorithm (layouts chosen so softmax/PV need no on-chip transposes):
  - Host passes Q^T, K^T as [D=128, S=2048] f32 (D on partitions) and V
    pre-arranged as VS [128, S] bf16 where column block j holds V rows
    [128j, 128j+128).
  - Scores are computed transposed, per key block j:
        S^T_j[k, q] = (K^T_j)-stationary.T @ Q^T-moving   (PSUM, f32)
    float32r matmuls (1 cycle/row for moving width >= 256) keep the scores
    at full precision (exp amplifies score error).
  - Causal mask applied additively on the PSUM scores (diagonal block only).
  - exp with the 1/sqrt(D) scale folded into ScalarE's activation affine,
    PSUM -> SBUF, output in bf16 (P^T tiles).
  - out^T[d, q] += V_j-stationary @ P^T_j-moving (bf16 in, f32 accumulate).
  - rowsum[q]  += ones-stationary @ P^T_j-moving (M=1 matmul, bf16).
  - Normalize per 512-wide q-chunk as soon as its accumulation finishes:
    evacuate out^T chunk, broadcast rowsum across partitions with a K=1
    matmul (reusing the freed out^T PSUM bank in place),
    reciprocal_approx_fast, multiply, DMA out.
  - Host transposes out^T back to [S, D].

TensorE work is software-pipelined: scores for key block j+1 are emitted
before PV/rowsum of block j so the PE never head-of-line blocks on ScalarE's
exp. The q axis is processed in two passes of 1024 so PSUM fits:
  staging S^T [128,1024] x2 bufs (4 banks) + out^T [128,1024] (2 banks)
  + 2x rowsum [1,512] (2 banks) = 8 banks.
"""

import math
import sys

import numpy as np
import ml_dtypes

sys.path.insert(0, "/opt/trn_rl_repo")

from concourse import bacc, mybir
from concourse.bass_utils import run_bass_kernel_spmd
from concourse.tile import TileContext

F32 = mybir.dt.float32
F32R = mybir.dt.float32r
BF16 = mybir.dt.bfloat16
BF16_NP = np.dtype(ml_dtypes.bfloat16)

B, S, D = 8, 2048, 128
NBLK = S // 128  # 16 key blocks
HALF = 1024  # q-pass width
SCALE = 1.0 / math.sqrt(D)
MASKNEG = -1e30

_NC_CACHE = None
_ONES = np.ones((128, 128), dtype=np.float32)


def _chunks_for_block(j, q0):
    """Matmul chunks for key block j in pass [q0, q0+HALF): list of
    (a, b, h) global q ranges clipped to psum bank h (bf16: no min width)."""
    k0 = 128 * j
    q_lo = max(q0, k0)
    out = []
    for h in range(2):
        a = max(q_lo, q0 + 512 * h)
        b = q0 + 512 * (h + 1)
        if a < b:
            out.append((a, b, h))
    return out


def _build_nc():
    nc = bacc.Bacc("TRN2", target_bir_lowering=False, debug=False, num_devices=8)

    qt_d = nc.dram_tensor("QT", [D, S], BF16, kind="ExternalInput")
    kt_d = nc.dram_tensor("KT", [D, S], BF16, kind="ExternalInput")
    vs_d = nc.dram_tensor("VS", [128, S], BF16, kind="ExternalInput")
    ones_d = nc.dram_tensor("ONES", [128, 128], F32R, kind="ExternalInput")
    out_d = nc.dram_tensor("out", [D, S], F32, kind="ExternalOutput")

    with TileContext(nc) as tc:
        with (
            tc.tile_pool(name="persist", bufs=1) as persist,
            tc.tile_pool(name="ptp", bufs=4) as ptp,
            tc.tile_pool(name="epi", bufs=2) as epi,
            tc.tile_pool(name="spool", bufs=2, space="PSUM") as spool,
            tc.tile_pool(name="opool", bufs=1, space="PSUM") as opool,
            tc.tile_pool(name="rpool", bufs=2, space="PSUM") as rpool,
        ):
            qt = persist.tile([D, S], BF16, tag="qt")
            kt = persist.tile([D, S], BF16, tag="kt")
            vs = persist.tile([128, S], BF16, tag="vs")  # col block j = V rows
            ones = persist.tile([128, 128], F32R, tag="ones")
            ones_r = ones[0:1, :]
            ones_b = persist.tile([128, 1], BF16, tag="ones_b")

            # additive causal mask, f32: strict-lower-triangle = MASKNEG
            mask = persist.tile([128, 128], F32, tag="mask")
            nc.gpsimd.memset(mask[:, :], 0.0)
            nc.gpsimd.affine_select(
                out=mask[:, :],
                in_=mask[:, :],
                compare_op=mybir.AluOpType.is_ge,
                fill=MASKNEG,
                base=0,
                pattern=[[1, 128]],
                channel_multiplier=-1,
            )

            # warm the ScalarE exp table while input DMAs run
            warm = epi.tile([1, 16], F32, tag="warm")
            nc.scalar.activation(
                warm[:, :],
                mask[0:1, 0:16],
                mybir.ActivationFunctionType.Exp,
                scale=SCALE,
            )

            # ---- input DMAs: critical prefix split across the SP hw queue
            # and gpsimd's software DGE; bulk on gpsimd in first-need order --
            nc.sync.dma_start(kt[:, 0:128], kt_d[:, 0:128])
            nc.sync.dma_start(qt[:, 0:256], qt_d[:, 0:256])
            nc.sync.dma_start(qt[:, 512:768], qt_d[:, 512:768])
            nc.sync.dma_start(vs[:, 0:512], vs_d[:, 0:512])
            nc.sync.dma_start(ones[:, :], ones_d[:, :])
            nc.gpsimd.dma_start(qt[:, 256:512], qt_d[:, 256:512])
            nc.gpsimd.dma_start(qt[:, 768:1024], qt_d[:, 768:1024])
            with nc.allow_low_precision(reason="exact ones to bf16"):
                nc.vector.tensor_copy(ones_b[:, :], ones[:, 0:1])
            for c in range(1, 8):  # kt blocks 1..7
                nc.gpsimd.dma_start(kt[:, c * 128 : (c + 1) * 128],
                                    kt_d[:, c * 128 : (c + 1) * 128])
            nc.gpsimd.dma_start(vs[:, 512:1024], vs_d[:, 512:1024])
            for c in range(4, 6):  # qt q in [1024, 1536)
                nc.gpsimd.dma_start(qt[:, c * 256 : (c + 1) * 256],
                                    qt_d[:, c * 256 : (c + 1) * 256])
            for c in range(8, 12):  # kt blocks 8..11
                nc.gpsimd.dma_start(kt[:, c * 128 : (c + 1) * 128],
                                    kt_d[:, c * 128 : (c + 1) * 128])
            nc.gpsimd.dma_start(vs[:, 1024:1536], vs_d[:, 1024:1536])
            for c in range(6, 8):  # qt q in [1536, 2048)
                nc.gpsimd.dma_start(qt[:, c * 256 : (c + 1) * 256],
                                    qt_d[:, c * 256 : (c + 1) * 256])
            for c in range(12, 16):  # kt blocks 12..15
                nc.gpsimd.dma_start(kt[:, c * 128 : (c + 1) * 128],
                                    kt_d[:, c * 128 : (c + 1) * 128])
            nc.gpsimd.dma_start(vs[:, 1536:2048], vs_d[:, 1536:2048])

            for qh in range(2):
                q0 = qh * HALF  # global q offset of this pass
                njb = (q0 + HALF) // 128  # key blocks this pass

                out_ps = opool.tile([D, HALF], F32, tag="outps",
                                    name=f"outps_{qh}")
                rs = [
                    rpool.tile([1, 512], F32, tag="rs", name=f"rs_{qh}_{h}")
                    for h in range(2)
                ]
                # last key block that touches each 512-half of out_ps
                j_last = [(q0 + 512 * (h + 1)) // 128 - 1 for h in range(2)]

                def emit_scores(j, qh=qh, q0=q0):
                    """QK^T chunks + mask + per-chunk exp for key block j.
                    exp is emitted per 512-chunk so PV of one chunk never
                    waits the other chunk's scores or the diagonal mask."""
                    k0 = 128 * j
                    chunks = _chunks_for_block(j, q0)

                    sps = spool.tile([128, HALF], F32, tag="sps",
                                     name=f"sps_{qh}_{j}")
                    pt = ptp.tile([128, HALF], BF16, tag="pt",
                                  name=f"pt_{qh}_{j}")
                    for (a, b, _h) in chunks:
                        if j == 0 and qh == 0 and a == 0:
                            # split the very first matmul so PE starts as
                            # soon as the first 256 qt columns arrive
                            nc.tensor.matmul(sps[:, 0:256], kt[:, 0:128],
                                             qt[:, 0:256], start=True,
                                             stop=True)
                            nc.tensor.matmul(sps[:, 256:512], kt[:, 0:128],
                                             qt[:, 256:512], start=True,
                                             stop=True)
                        else:
                            nc.tensor.matmul(
                                sps[:, a - q0 : b - q0],
                                kt[:, k0 : k0 + 128],
                                qt[:, a:b],
                                start=True,
                                stop=True,
                            )
                    if k0 >= q0:
                        dl = k0 - q0
                        nc.vector.tensor_add(
                            sps[:, dl : dl + 128],
                            sps[:, dl : dl + 128],
                            mask[:, :],
                        )
                    lo = chunks[0][0]
                    nc.scalar.activation(
                        pt[:, lo - q0 : HALF],
                        sps[:, lo - q0 : HALF],
                        mybir.ActivationFunctionType.Exp,
                        scale=SCALE,
                    )
                    return pt

                def emit_pv(j, pt, q0=q0):
                    """PV + rowsum accumulation for key block j. Chunks are
                    grouped per stationary operand: consecutive matmuls with
                    the same weights skip the ~107ns weight-reload stall."""
                    k0 = 128 * j
                    chunks = _chunks_for_block(j, q0)
                    for (a, b, h) in chunks:
                        nc.tensor.matmul(
                            out_ps[:, a - q0 : b - q0],
                            vs[:, k0 : k0 + 128],
                            pt[:, a - q0 : b - q0],
                            start=(j == 0),
                            stop=(j == j_last[h]),
                        )
                    for (a, b, h) in chunks:
                        nc.tensor.matmul(
                            rs[h][:, a - (q0 + 512 * h) : b - (q0 + 512 * h)],
                            ones_b[:, :],
                            pt[:, a - q0 : b - q0],
                            start=(j == 0),
                            stop=(j == j_last[h]),
                        )

                def emit_epi_half(h, qh=qh, q0=q0):
                    """Normalize + store q-chunk [q0+512h, q0+512h+512)."""
                    sl = slice(512 * h, 512 * (h + 1))
                    rs_row = epi.tile([1, 512], F32R, tag="rs_row",
                                      name=f"rsr_{qh}_{h}")
                    with nc.allow_low_precision(reason="f32r softmax denom"):
                        nc.vector.tensor_copy(rs_row[:, :], rs[h][:, :])
                    o_evac = epi.tile([D, 512], F32, tag="o_evac",
                                      name=f"oev_{qh}_{h}")
                    nc.vector.tensor_copy(o_evac[:, :], out_ps[:, sl])
                    # broadcast rowsum into the just-freed out_ps bank
                    nc.tensor.matmul(
                        out_ps[:, sl],
                        ones_r[:, :],
                        rs_row[:, :],
                        start=True,
                        stop=True,
                    )
                    rb = epi.tile([128, 512], F32, tag="rb",
                                  name=f"rb_{qh}_{h}")
                    nc.vector.reciprocal_approx_fast(
                        out=rb[:, :], in_=out_ps[:, sl]
                    )
                    o_fin = epi.tile([D, 512], F32, tag="o_fin",
                                     name=f"ofin_{qh}_{h}")
                    nc.vector.tensor_mul(o_fin[:, :], o_evac[:, :], rb[:, :])
                    for c in range(2):
                        nc.sync.dma_start(
                            out_d[:, q0 + 512 * h + 256 * c :
                                  q0 + 512 * h + 256 * (c + 1)],
                            o_fin[:, 256 * c : 256 * (c + 1)],
                        )

                # software pipeline: scores for j+1 before PV of j;
                # per-half epilogue as soon as its accumulation completes
                pts = {0: emit_scores(0)}
                for j in range(njb):
                    if j + 1 < njb:
                        pts[j + 1] = emit_scores(j + 1)
                    emit_pv(j, pts.pop(j))
                    for h in range(2):
                        if j == j_last[h]:
                            emit_epi_half(h)

    nc.compile()
    return nc


def _get_nc():
    global _NC_CACHE
    if _NC_CACHE is None:
        _NC_CACHE = _build_nc()
    return _NC_CACHE


def _in_maps(Q, K, V):
    maps = []
    for b in range(B):
        vsb = np.ascontiguousarray(
            V[b].reshape(NBLK, 128, D).transpose(1, 0, 2).reshape(128, S)
        ).astype(BF16_NP)
        maps.append(
            {
                "QT": np.ascontiguousarray(Q[b].T).astype(BF16_NP),
                "KT": np.ascontiguousarray(K[b].T).astype(BF16_NP),
                "VS": vsb,
                "ONES": _ONES,
            }
        )
    return maps


def kernel(Q, K, V):
    Q = np.asarray(Q, dtype=np.float32)
    K = np.asarray(K, dtype=np.float32)
    V = np.asarray(V, dtype=np.float32)
    assert Q.shape == (B, S, D), Q.shape

    nc = _get_nc()
    res = run_bass_kernel_spmd(nc, _in_maps(Q, K, V), core_ids=list(range(B)))
    return np.stack(
        [np.ascontiguousarray(res.results[b]["out"].T) for b in range(B)], axis=0
    )
